# revision 10
# baseline (speedup 1.0000x reference)
"""Trainium2 Bass kernel for nn_CIGA (GCN + edge-attention top-k split).

Self-contained: host shards/preprocesses indices, device (8 NeuronCores)
does all tensor compute via fp32 matmuls (exact), ACT relu-reductions and
an on-device per-graph float bisection for the exact top-k threshold.
"""
import math
import numpy as np

import concourse.bass as bass
import concourse.bacc as bacc
import concourse.mybir as mybir
import concourse.tile as tile
from concourse.bass_utils import run_bass_kernel_spmd
from concourse.masks import make_identity

f32 = mybir.dt.float32

N = 50000
E = 800000
G = 128
H = 64
NCORES = 8
GPC = G // NCORES  # graphs per core
RATIO = 0.25
P = 128

_cache = {}
LAST_DEVICE_WALLS = []


def _build_gcn_prog(E_PAD, NT_LOC, T_U):
    """One GCN layer: h' = relu((S@msgs + diag(invdeg)@hloc) @ W + sigma x b).

    S carries norm_e in a one-hot layout (host built), msgs = h[row] gathered
    on host. All matmuls fp32 (exact 4-pass).
    """
    nc = bacc.Bacc("TRN2", target_bir_lowering=False)
    msgs_in = nc.dram_tensor("msgs", [E_PAD, H], f32, kind="ExternalInput")
    s_in = nc.dram_tensor("s", [E_PAD, P], f32, kind="ExternalInput")
    hloc_in = nc.dram_tensor("hloc", [NT_LOC * P, H], f32, kind="ExternalInput")
    diag_in = nc.dram_tensor("diag", [NT_LOC * P, P], f32, kind="ExternalInput")
    sig_in = nc.dram_tensor("sig", [1, NT_LOC * P], f32, kind="ExternalInput")
    w_in = nc.dram_tensor("w", [H, H], f32, kind="ExternalInput")
    b_in = nc.dram_tensor("b", [1, H], f32, kind="ExternalInput")
    hout = nc.dram_tensor("hout", [NT_LOC * P, H], f32, kind="ExternalOutput")

    EPT = T_U * P  # edges per dest tile
    with tile.TileContext(nc) as tc:
        with (
            tc.tile_pool(name="const", bufs=1) as cpool,
            tc.tile_pool(name="stream", bufs=3) as spool,
            tc.tile_pool(name="node", bufs=3) as npool,
            tc.tile_pool(name="ps", bufs=2, space="PSUM") as ps,
            tc.tile_pool(name="ps2", bufs=2, space="PSUM") as ps2,
        ):
            wt = cpool.tile([H, H], f32)
            nc.sync.dma_start(wt[:], w_in[:])
            bt = cpool.tile([1, H], f32)
            nc.sync.dma_start(bt[:], b_in[:])
            sigt = cpool.tile([1, NT_LOC * P], f32)
            nc.sync.dma_start(sigt[:], sig_in[:])
            ident = cpool.tile([P, P], f32)
            make_identity(nc, ident[:])

            for d in range(NT_LOC):
                mt = spool.tile([P, T_U * H], f32, tag="mt")
                nc.sync.dma_start(
                    mt[:].rearrange("p (n h) -> p n h", h=H),
                    msgs_in.ap()[d * EPT:(d + 1) * EPT, :].rearrange(
                        "(n p) h -> p n h", p=P),
                )
                st = spool.tile([P, T_U * P], f32, tag="st")
                nc.sync.dma_start(
                    st[:].rearrange("p (n h) -> p n h", h=P),
                    s_in.ap()[d * EPT:(d + 1) * EPT, :].rearrange(
                        "(n p) h -> p n h", p=P),
                )
                ht = npool.tile([P, H], f32, tag="ht")
                nc.sync.dma_start(ht[:], hloc_in.ap()[d * P:(d + 1) * P, :])
                dt = npool.tile([P, P], f32, tag="dt")
                nc.sync.dma_start(dt[:], diag_in.ap()[d * P:(d + 1) * P, :])

                agg = ps.tile([P, H], f32)
                for t in range(T_U):
                    nc.tensor.matmul(
                        agg[:], st[:, t * P:(t + 1) * P], mt[:, t * H:(t + 1) * H],
                        start=(t == 0), stop=False)
                nc.tensor.matmul(agg[:], dt[:], ht[:], start=False, stop=True)
                aggs = npool.tile([P, H], f32, tag="aggs")
                nc.vector.tensor_copy(aggs[:], agg[:])
                # transpose agg -> [H, P]
                aggT_ps = ps2.tile([H, P], f32, tag="aggT")
                nc.tensor.transpose(aggT_ps[:], aggs[:], ident[:])
                aggT = npool.tile([H, P], f32, tag="aggT_s")
                nc.vector.tensor_copy(aggT[:], aggT_ps[:])
                hn = ps.tile([P, H], f32, tag="hn")
                nc.tensor.matmul(hn[:], aggT[:], wt[:], start=True, stop=False)
                nc.tensor.matmul(hn[:], sigt[:, d * P:(d + 1) * P], bt[:],
                                 start=False, stop=True)
                ho = npool.tile([P, H], f32, tag="ho")
                nc.scalar.activation(ho[:], hn[:],
                                     mybir.ActivationFunctionType.Relu)
                nc.sync.dma_start(hout.ap()[d * P:(d + 1) * P, :], ho[:])
    nc.compile()
    return nc


def _build_mlp_prog(T_G, P_pos, BIS_ROUNDS=36):
    """Edge MLP scores + per-graph exact top-k threshold via float bisection.

    ert [128=2H, E2] host-transposed gathered features; score per edge =
    sum(relu(z[:, :P_pos])) - sum(relu(z[:, P_pos:])) + ba2 with
    z = ert_tile.T @ Wa1p (Wa1p = Wa1 * |Wa2| col-permuted).
    """
    NTI = GPC * T_G  # edge tiles per core
    E2 = NTI * P
    nc = bacc.Bacc("TRN2", target_bir_lowering=False)
    ert_in = nc.dram_tensor("ert", [2 * H, E2], f32, kind="ExternalInput")
    wa1_in = nc.dram_tensor("wa1p", [2 * H, 4 * H], f32, kind="ExternalInput")
    ba1_in = nc.dram_tensor("ba1p", [1, 4 * H], f32, kind="ExternalInput")
    smask_in = nc.dram_tensor("smask", [P, NTI], f32, kind="ExternalInput")
    k_in = nc.dram_tensor("ktile", [1, GPC], f32, kind="ExternalInput")
    ba2_in = nc.dram_tensor("ba2", [P, 1], f32, kind="ExternalInput")
    score_out = nc.dram_tensor("score", [P, NTI], f32, kind="ExternalOutput")
    causal_out = nc.dram_tensor("causal", [P, NTI], f32, kind="ExternalOutput")
    spu_out = nc.dram_tensor("spu", [P, NTI], f32, kind="ExternalOutput")
    mask_out = nc.dram_tensor("maskf", [P, NTI], f32, kind="ExternalOutput")
    thr_out = nc.dram_tensor("thr", [1, GPC], f32, kind="ExternalOutput")

    HH = 4 * H  # 256
    with tile.TileContext(nc) as tc:
        with (
            tc.tile_pool(name="const", bufs=1) as cpool,
            tc.tile_pool(name="stream", bufs=4) as spool,
            tc.tile_pool(name="acc", bufs=1) as apool,
            tc.tile_pool(name="ps", bufs=4, space="PSUM") as ps,
            tc.tile_pool(name="ps2", bufs=1, space="PSUM") as ps2,
            tc.tile_pool(name="tiny", bufs=2) as tpool,
        ):
            wa1 = cpool.tile([2 * H, HH], f32)
            nc.sync.dma_start(wa1[:], wa1_in[:])
            ba1 = cpool.tile([1, HH], f32)
            nc.sync.dma_start(ba1[:], ba1_in[:])
            ones_r = cpool.tile([1, P], f32)
            nc.vector.memset(ones_r[:], 1.0)
            ones_c = cpool.tile([P, 1], f32)
            nc.vector.memset(ones_c[:], 1.0)
            ba2t = cpool.tile([P, 1], f32)
            nc.sync.dma_start(ba2t[:], ba2_in[:])
            ktile = cpool.tile([1, GPC], f32)
            nc.sync.dma_start(ktile[:], k_in[:])
            smask = cpool.tile([P, NTI], f32)
            nc.sync.dma_start(smask[:], smask_in[:])

            posb = apool.tile([P, NTI], f32)
            negb = apool.tile([P, NTI], f32)

            CH = 4  # edge tiles per DMA chunk
            for c0 in range(0, NTI, CH):
                cw = min(CH, NTI - c0)
                et = spool.tile([2 * H, CH * P], f32, tag="et")
                nc.sync.dma_start(et[:, :cw * P], ert_in.ap()[:, c0 * P:(c0 + cw) * P])
                for j in range(cw):
                    t = c0 + j
                    z = ps.tile([P, HH], f32, tag="z")
                    # z[e, hid] = ert_tile.T @ Wa1p ; add bias via K=1 matmul
                    nc.tensor.matmul(z[:], et[:, j * P:(j + 1) * P], wa1[:],
                                     start=True, stop=False)
                    nc.tensor.matmul(z[:], ones_r[:], ba1[:], start=False,
                                     stop=True)
                    nc.scalar.activation(
                        spool.tile([P, P_pos], f32, tag="junkp", name="junkp")[:],
                        z[:, :P_pos],
                        mybir.ActivationFunctionType.Relu,
                        accum_out=posb[:, t:t + 1])
                    nc.scalar.activation(
                        spool.tile([P, HH - P_pos], f32, tag="junkn", name="junkn")[:],
                        z[:, P_pos:],
                        mybir.ActivationFunctionType.Relu,
                        accum_out=negb[:, t:t + 1])

            scores = apool.tile([P, NTI], f32)
            nc.vector.tensor_tensor(scores[:], posb[:], negb[:],
                                    op=mybir.AluOpType.subtract)
            nc.vector.tensor_scalar(scores[:], scores[:], ba2t[:, :1], None,
                                    mybir.AluOpType.add)
            # masked keys: pad -> -1e30 (for counting) / +1e30 (for min)
            keys = apool.tile([P, NTI], f32)
            nc.vector.tensor_tensor(keys[:], scores[:], smask[:],
                                    op=mybir.AluOpType.mult)
            inv = apool.tile([P, NTI], f32)
            nc.vector.tensor_scalar(inv[:], smask[:], -1.0, 1.0,
                                    mybir.AluOpType.add,
                                    mybir.AluOpType.mult)  # (smask-1)*1? see below
            # inv = (smask - 1.0) * 1.0 -> 0 for valid, -1 for pad
            keylo = apool.tile([P, NTI], f32)  # pad -> +1e30
            nc.vector.tensor_scalar(keylo[:], inv[:], -1e30, None,
                                    mybir.AluOpType.mult)
            nc.vector.tensor_tensor(keylo[:], keys[:], keylo[:],
                                    op=mybir.AluOpType.add)
            keyhi = apool.tile([P, NTI], f32)  # pad -> -1e30
            nc.vector.tensor_scalar(keyhi[:], inv[:], 1e30, None,
                                    mybir.AluOpType.mult)
            nc.vector.tensor_tensor(keyhi[:], keys[:], keyhi[:],
                                    op=mybir.AluOpType.add)

            # global min/max over valid scores
            ident = cpool.tile([P, P], f32)
            make_identity(nc, ident[:])
            redmin = tpool.tile([P, 1], f32, tag="redmin")
            nc.vector.tensor_reduce(redmin[:], keylo[:],
                                    mybir.AxisListType.X, mybir.AluOpType.min)
            redmax = tpool.tile([P, 1], f32, tag="redmax")
            nc.vector.tensor_reduce(redmax[:], keyhi[:],
                                    mybir.AxisListType.X, mybir.AluOpType.max)
            rminT_ps = ps2.tile([1, P], f32, tag="small")
            nc.tensor.transpose(rminT_ps[:], redmin[:], ident[:])
            rminT = tpool.tile([1, P], f32, tag="rminT")
            nc.vector.tensor_copy(rminT[:], rminT_ps[:])
            rmaxT_ps = ps2.tile([1, P], f32, tag="small")
            nc.tensor.transpose(rmaxT_ps[:], redmax[:], ident[:])
            rmaxT = tpool.tile([1, P], f32, tag="rmaxT")
            nc.vector.tensor_copy(rmaxT[:], rmaxT_ps[:])
            mmin = tpool.tile([1, 1], f32, tag="mmin")
            nc.vector.tensor_reduce(mmin[:], rminT[:],
                                    mybir.AxisListType.X, mybir.AluOpType.min)
            mmax = tpool.tile([1, 1], f32, tag="mmax")
            nc.vector.tensor_reduce(mmax[:], rmaxT[:],
                                    mybir.AxisListType.X, mybir.AluOpType.max)
            lo = tpool.tile([1, GPC], f32, tag="lo")
            hi = tpool.tile([1, GPC], f32, tag="hi")
            nc.vector.tensor_copy(lo[:], mmin[:].to_broadcast([1, GPC]))
            nc.vector.tensor_copy(hi[:], mmax[:].to_broadcast([1, GPC]))
            # widen a touch
            span = tpool.tile([1, GPC], f32, tag="span")
            nc.vector.tensor_tensor(span[:], hi[:], lo[:],
                                    op=mybir.AluOpType.subtract)
            nc.vector.tensor_scalar(span[:], span[:], 0.001, None,
                                    mybir.AluOpType.mult)
            nc.vector.tensor_tensor(lo[:], lo[:], span[:],
                                    op=mybir.AluOpType.subtract)
            nc.vector.tensor_tensor(hi[:], hi[:], span[:],
                                    op=mybir.AluOpType.add)

            cnt = tpool.tile([P, GPC], f32, tag="cnt")
            for r in range(BIS_ROUNDS):
                cand = tpool.tile([1, GPC], f32, tag="cand")
                nc.vector.tensor_tensor(cand[:], lo[:], hi[:],
                                        op=mybir.AluOpType.add)
                nc.vector.tensor_scalar(cand[:], cand[:], 0.5, None,
                                        mybir.AluOpType.mult)
                # broadcast cand down partitions: [128, GPC]
                cb_ps = ps2.tile([P, GPC], f32, tag="small")
                nc.tensor.matmul(cb_ps[:], ones_r[:], cand[:], start=True,
                                 stop=True)
                cb = tpool.tile([P, GPC], f32, tag="cbs")
                nc.vector.tensor_copy(cb[:], cb_ps[:])
                for g in range(GPC):
                    nc.vector.tensor_scalar(
                        tpool.tile([P, T_G], f32, tag="junkc", name="junkc")[:],
                        keyhi[:, g * T_G:(g + 1) * T_G],
                        cb[:, g:g + 1], 0.0, mybir.AluOpType.is_ge,
                        mybir.AluOpType.add,
                        accum_out=cnt[:, g:g + 1])
                tot_ps = ps2.tile([1, GPC], f32, tag="small")
                nc.tensor.matmul(tot_ps[:], ones_c[:], cnt[:], start=True, stop=True)
                tot = tpool.tile([1, GPC], f32, tag="tots")
                nc.vector.tensor_copy(tot[:], tot_ps[:])
                dlt = tpool.tile([1, GPC], f32, tag="dlt")
                nc.vector.tensor_tensor(dlt[:], tot[:], ktile[:],
                                        op=mybir.AluOpType.is_ge)
                # lo += d*(cand-lo); hi = cand + d*(hi-cand)
                d1 = tpool.tile([1, GPC], f32, tag="d1")
                nc.vector.tensor_tensor(d1[:], cand[:], lo[:],
                                        op=mybir.AluOpType.subtract)
                nc.vector.tensor_tensor(d1[:], d1[:], dlt[:],
                                        op=mybir.AluOpType.mult)
                nc.vector.tensor_tensor(lo[:], lo[:], d1[:],
                                        op=mybir.AluOpType.add)
                d2 = tpool.tile([1, GPC], f32, tag="d2")
                nc.vector.tensor_tensor(d2[:], hi[:], cand[:],
                                        op=mybir.AluOpType.subtract)
                nc.vector.tensor_tensor(d2[:], d2[:], dlt[:],
                                        op=mybir.AluOpType.mult)
                nc.vector.tensor_tensor(hi[:], cand[:], d2[:],
                                        op=mybir.AluOpType.add)

            nc.sync.dma_start(thr_out[:], lo[:])
            # final mask and outputs
            lob_ps = ps2.tile([P, GPC], f32, tag="small")
            nc.tensor.matmul(lob_ps[:], ones_r[:], lo[:], start=True, stop=True)
            lob = tpool.tile([P, GPC], f32, tag="lobs")
            nc.vector.tensor_copy(lob[:], lob_ps[:])
            maskt = apool.tile([P, NTI], f32)
            for g in range(GPC):
                nc.vector.tensor_scalar(maskt[:, g * T_G:(g + 1) * T_G],
                                        keyhi[:, g * T_G:(g + 1) * T_G],
                                        lob[:, g:g + 1], None,
                                        mybir.AluOpType.is_ge)
            causal = apool.tile([P, NTI], f32)
            nc.vector.tensor_tensor(causal[:], scores[:], maskt[:],
                                    op=mybir.AluOpType.mult)
            spu = apool.tile([P, NTI], f32)
            nc.vector.tensor_tensor(spu[:], causal[:], scores[:],
                                    op=mybir.AluOpType.subtract)
            nc.sync.dma_start(score_out[:], scores[:])
            nc.sync.dma_start(causal_out[:], causal[:])
            nc.sync.dma_start(spu_out[:], spu[:])
            nc.sync.dma_start(mask_out[:], maskt[:])
    nc.compile()
    return nc


def kernel(x, edge_index, batch, W1, b1, W2, b2, W3, b3, Wa1, ba1, Wa2, ba2):
    x = np.ascontiguousarray(np.asarray(x, np.float32))
    ei = np.asarray(edge_index)
    batch = np.asarray(batch)
    row = ei[0].astype(np.int64)
    col = ei[1].astype(np.int64)
    batch64 = batch.astype(np.int64)
    W1, b1, W2, b2, W3, b3 = [np.asarray(a, np.float32) for a in (W1, b1, W2, b2, W3, b3)]
    Wa1 = np.asarray(Wa1, np.float32)
    ba1 = np.asarray(ba1, np.float32)
    Wa2 = np.asarray(Wa2, np.float32).reshape(-1)
    ba2 = np.float32(np.asarray(ba2).reshape(())) if np.asarray(ba2).size else np.float32(0)

    # ---------------- host index preprocessing ----------------
    deg = (1.0 + np.bincount(col, minlength=N)).astype(np.float32)
    a_rs = (1.0 / np.sqrt(deg)).astype(np.float32)
    inv_deg = (np.float32(1.0) / deg).astype(np.float32)
    norm_e = (a_rs[row] * a_rs[col]).astype(np.float32)

    # node ranges per core (batch sorted)
    nb = np.searchsorted(batch64, np.arange(0, G + 1, GPC))  # [9]
    n_loc = nb[1:] - nb[:-1]
    NT_LOC = int(math.ceil(n_loc.max() / P))
    NLOC_PAD = NT_LOC * P

    # GCN edge shard by col-owner, grouped by dest tile
    core_of_col = np.searchsorted(nb[1:], col, side="right")
    order = np.argsort(col, kind="stable")
    T_U = 0
    gcn_meta = []
    for c in range(NCORES):
        sel = order[core_of_col[order] == c]
        lcol = col[sel] - nb[c]
        dtile = lcol // P
        cnts = np.bincount(dtile, minlength=NT_LOC)
        T_U = max(T_U, int(math.ceil(cnts.max() / P)))
        gcn_meta.append((sel, dtile, cnts))
    EPT = T_U * P
    E_PAD = NT_LOC * EPT

    gcn_rows = []
    gcn_S = []
    gcn_diag = []
    gcn_sig = []
    for c in range(NCORES):
        sel, dtile, cnts = gcn_meta[c]
        rows_c = np.zeros(E_PAD, np.int64)
        S_c = np.zeros((E_PAD, P), np.float32)
        off = np.concatenate([[0], np.cumsum(cnts)])
        for d in range(NT_LOC):
            seg = sel[off[d]:off[d + 1]]
            m = len(seg)
            basep = d * EPT
            rows_c[basep:basep + m] = row[seg]
            S_c[np.arange(basep, basep + m), (col[seg] - nb[c]) - d * P] = norm_e[seg]
        gcn_rows.append(rows_c)
        gcn_S.append(S_c)
        dg = np.zeros((NLOC_PAD, P), np.float32)
        nl = int(n_loc[c])
        idx = np.arange(nl)
        dg[idx, idx % P] = inv_deg[nb[c]:nb[c + 1]]
        gcn_diag.append(dg)
        sg = np.zeros((1, NLOC_PAD), np.float32)
        # sigma_v = sum_e norm_e + inv_deg_v
        ssum = np.bincount(col, weights=norm_e.astype(np.float64), minlength=N).astype(np.float32)
        sg[0, :nl] = ssum[nb[c]:nb[c + 1]] + inv_deg[nb[c]:nb[c + 1]]
        gcn_sig.append(sg)

    # MLP edge shard by graph of row
    grp = batch64[row]
    gorder = np.argsort(grp, kind="stable")
    gcnts = np.bincount(grp, minlength=G)
    goff = np.concatenate([[0], np.cumsum(gcnts)])
    T_G = int(math.ceil(gcnts.max() / P))
    NTI = GPC * T_G
    E2 = NTI * P
    k_g = np.ceil(RATIO * gcnts.astype(np.float64)).astype(np.int64)

    mlp_rows = []
    mlp_cols = []
    mlp_mask = []
    mlp_k = []
    mlp_edge_ids = []
    for c in range(NCORES):
        rows_c = np.zeros(E2, np.int64)
        cols_c = np.zeros(E2, np.int64)
        msk = np.zeros(E2, np.float32)
        eids = np.full(E2, -1, np.int64)
        for gi in range(GPC):
            g = c * GPC + gi
            seg = gorder[goff[g]:goff[g + 1]]
            m = len(seg)
            basep = gi * T_G * P
            rows_c[basep:basep + m] = row[seg]
            cols_c[basep:basep + m] = col[seg]
            msk[basep:basep + m] = 1.0
            eids[basep:basep + m] = seg
        mlp_rows.append(rows_c)
        mlp_cols.append(cols_c)
        # [P, NTI] layout: edge (tile t, part p) = linear t*P+p
        mlp_mask.append(msk.reshape(NTI, P).T.copy())
        mlp_k.append(k_g[c * GPC:(c + 1) * GPC].astype(np.float32).reshape(1, GPC))
        mlp_edge_ids.append(eids)

    # Wa2 sign-fold into Wa1/ba1
    absw = np.abs(Wa2).astype(np.float32)
    perm = np.argsort(Wa2 <= 0, kind="stable")  # positives first
    P_pos = int((Wa2 > 0).sum())
    Wa1p = (Wa1 * absw[None, :]).astype(np.float32)[:, perm].copy()
    ba1p = (ba1 * absw).astype(np.float32)[perm].reshape(1, -1).copy()

    # ---------------- programs (cached across calls) ----------------
    key = (E_PAD, NT_LOC, T_U, T_G, P_pos)
    if key not in _cache:
        _cache[key] = (
            _build_gcn_prog(E_PAD, NT_LOC, T_U),
            _build_mlp_prog(T_G, P_pos),
        )
    gcn_nc, mlp_nc = _cache[key]

    # ---------------- run: 3 GCN layers ----------------
    import time as _time
    LAST_DEVICE_WALLS.clear()
    h = x
    for (W, b) in ((W1, b1), (W2, b2), (W3, b3)):
        maps = []
        for c in range(NCORES):
            hloc = np.zeros((NLOC_PAD, H), np.float32)
            hloc[:n_loc[c]] = h[nb[c]:nb[c + 1]]
            maps.append({
                "msgs": np.ascontiguousarray(h[gcn_rows[c]]),
                "s": gcn_S[c],
                "hloc": hloc,
                "diag": gcn_diag[c],
                "sig": gcn_sig[c],
                "w": W,
                "b": b.reshape(1, H),
            })
        _t0 = _time.time()
        res = run_bass_kernel_spmd(gcn_nc, maps, core_ids=list(range(NCORES)))
        LAST_DEVICE_WALLS.append(_time.time() - _t0)
        h = np.concatenate(
            [res.results[c]["hout"][:n_loc[c]] for c in range(NCORES)], 0)

    # ---------------- run: MLP + topk ----------------
    maps = []
    for c in range(NCORES):
        ert = np.zeros((2 * H, E2), np.float32)
        ert[:H] = h[mlp_rows[c]].T
        ert[H:] = h[mlp_cols[c]].T
        pad = mlp_mask[c].T.reshape(-1) == 0.0  # linear order
        if pad.any():
            ert[:, pad] = 0.0
        maps.append({
            "ert": ert,
            "wa1p": Wa1p,
            "ba1p": ba1p,
            "smask": mlp_mask[c],
            "ktile": mlp_k[c],
            "ba2": np.full((P, 1), ba2, np.float32),
        })
    _t0 = _time.time()
    res = run_bass_kernel_spmd(mlp_nc, maps, core_ids=list(range(NCORES)))
    LAST_DEVICE_WALLS.append(_time.time() - _t0)

    out = np.zeros((3, E), np.float32)
    mask = np.zeros(E, bool)
    for c in range(NCORES):
        r = res.results[c]
        eids = mlp_edge_ids[c]
        valid = eids >= 0
        ev = eids[valid]
        # [P, NTI] -> linear (t*P+p)
        sc = r["score"].T.reshape(-1)[valid]
        ca = r["causal"].T.reshape(-1)[valid]
        sp = r["spu"].T.reshape(-1)[valid]
        mk = r["maskf"].T.reshape(-1)[valid]
        out[0, ev] = sc
        out[1, ev] = ca
        out[2, ev] = sp
        mask[ev] = mk > 0.5
    return out, mask
